# revision 1
# baseline (speedup 1.0000x reference)
"""MoE (8 experts, top-2) Trainium2 Bass kernel, expert-parallel over 8 cores.

Pipeline (all FLOPs on device):
  L1: gating logits for all tokens (data-parallel over cores)
  host: top-2 selection + per-expert dispatch lists (comparisons/indexing only)
  L2: per-core expert MLP (3 layers, fp32) on gathered tokens, feature-major
  L3: per-token gather of its two expert rows + on-device softmax combine
"""

import numpy as np

import jax

jax.config.update("jax_compilation_cache_dir", "/tmp/jax_comp_cache")
jax.config.update("jax_persistent_cache_min_entry_size_bytes", -1)
jax.config.update("jax_persistent_cache_min_compile_time_secs", 0)

import concourse.bass as bass
import concourse.mybir as mybir
import concourse.tile as tile
from concourse import bacc
from concourse.bass_utils import run_bass_kernel_spmd
from concourse.kernels.tile_matmul import matmul_tile_kernel

N, D, H, O, E = 8192, 1024, 2048, 1024, 8
NCORES = 8
TPC = N // NCORES  # tokens per core (gating / combine phases)
F32 = mybir.dt.float32

_CACHE = {}


def _to_pmn(a):
    """[K, N] row-major -> [128, K/128, N] with row k = m*128 + p."""
    K, Nn = a.shape
    return np.ascontiguousarray(a.reshape(K // 128, 128, Nn).transpose(1, 0, 2))


def _nc():
    return bacc.Bacc(None, target_bir_lowering=False, debug=True)


def _build_gate_nc():
    nc = _nc()
    xt = nc.dram_tensor("xt", [128, D // 128, TPC], F32, kind="ExternalInput")
    wg1 = nc.dram_tensor("wg1", [128, D // 128, 128], F32, kind="ExternalInput")
    wg2 = nc.dram_tensor("wg2", [128, 1, 128], F32, kind="ExternalInput")
    g1t = nc.dram_tensor("g1t", [128, 1, TPC], F32)
    logt = nc.dram_tensor("logt", [128, 1, TPC], F32, kind="ExternalOutput")
    with tile.TileContext(nc) as tc:
        matmul_tile_kernel(tc, wg1[:], xt[:], g1t[:], use_relu=True)
        matmul_tile_kernel(tc, wg2[:], g1t[:], logt[:])
    nc.compile()
    return nc


def _build_mlp_nc(C):
    # Matmuls in bf16 (1 cycle/row on PE vs 4 for strict fp32), fp32 PSUM
    # accumulate. Weights/x cast to bf16 on device per tile; h1/h2 stored
    # bf16 (halves intermediate HBM traffic), final output fp32.
    BF = mybir.dt.bfloat16
    nc = _nc()
    xt = nc.dram_tensor("xt", [128, D // 128, C], F32, kind="ExternalInput")
    w1 = nc.dram_tensor("w1", [128, D // 128, H], F32, kind="ExternalInput")
    w2 = nc.dram_tensor("w2", [128, H // 128, H], F32, kind="ExternalInput")
    w3 = nc.dram_tensor("w3", [128, H // 128, O], F32, kind="ExternalInput")
    h1 = nc.dram_tensor("h1", [128, H // 128, C], BF)
    h2 = nc.dram_tensor("h2", [128, H // 128, C], BF)
    yt = nc.dram_tensor("yt", [128, O // 128, C], F32, kind="ExternalOutput")
    with tile.TileContext(nc) as tc:
        matmul_tile_kernel(tc, w1[:], xt[:], h1[:], use_relu=True, matmul_dtype=BF)
        matmul_tile_kernel(tc, w2[:], h1[:], h2[:], use_relu=True, matmul_dtype=BF)
        matmul_tile_kernel(tc, w3[:], h2[:], yt[:], matmul_dtype=BF)
    nc.compile()
    return nc


def _build_mlp2_nc(S):
    # Two independent expert-segment slots in one module: one launch does
    # the work of two, paying the kernel tail barrier/warmup once.
    BF = mybir.dt.bfloat16
    nc = _nc()
    t = {}
    for s in ("A", "B"):
        t[f"xt{s}"] = nc.dram_tensor(f"xt{s}", [128, D // 128, S], F32, kind="ExternalInput")
        t[f"w1{s}"] = nc.dram_tensor(f"w1{s}", [128, D // 128, H], F32, kind="ExternalInput")
        t[f"w2{s}"] = nc.dram_tensor(f"w2{s}", [128, H // 128, H], F32, kind="ExternalInput")
        t[f"w3{s}"] = nc.dram_tensor(f"w3{s}", [128, H // 128, O], F32, kind="ExternalInput")
        t[f"h1{s}"] = nc.dram_tensor(f"h1{s}", [128, H // 128, S], BF)
        t[f"h2{s}"] = nc.dram_tensor(f"h2{s}", [128, H // 128, S], BF)
        t[f"yt{s}"] = nc.dram_tensor(f"yt{s}", [128, O // 128, S], F32, kind="ExternalOutput")
    with tile.TileContext(nc) as tc:
        for s in ("A", "B"):
            matmul_tile_kernel(tc, t[f"w1{s}"][:], t[f"xt{s}"][:], t[f"h1{s}"][:], use_relu=True, matmul_dtype=BF)
            matmul_tile_kernel(tc, t[f"w2{s}"][:], t[f"h1{s}"][:], t[f"h2{s}"][:], use_relu=True, matmul_dtype=BF)
            matmul_tile_kernel(tc, t[f"w3{s}"][:], t[f"h2{s}"][:], t[f"yt{s}"][:], matmul_dtype=BF)
    nc.compile()
    return nc


def _build_combine_nc(yall_rows):
    nc = _nc()
    ntiles = TPC // 128
    yall = nc.dram_tensor("yall", [yall_rows, O], F32, kind="ExternalInput")
    lg = nc.dram_tensor("lg", [128, ntiles, E], F32, kind="ExternalInput")
    m0 = nc.dram_tensor("m0", [128, ntiles, E], F32, kind="ExternalInput")
    m1 = nc.dram_tensor("m1", [128, ntiles, E], F32, kind="ExternalInput")
    i0 = nc.dram_tensor("i0", [128, ntiles], mybir.dt.int32, kind="ExternalInput")
    i1 = nc.dram_tensor("i1", [128, ntiles], mybir.dt.int32, kind="ExternalInput")
    out = nc.dram_tensor("out", [128, ntiles, O], F32, kind="ExternalOutput")
    X = mybir.AxisListType.X
    with tile.TileContext(nc) as tc:
        with (
            tc.tile_pool(name="big", bufs=4) as gp,
            tc.tile_pool(name="small", bufs=8) as sp,
            tc.tile_pool(name="idx", bufs=4) as ip,
        ):
            for i in range(ntiles):
                lg_t = sp.tile([128, E], F32, tag="lg")
                m0_t = sp.tile([128, E], F32, tag="m0")
                m1_t = sp.tile([128, E], F32, tag="m1")
                i0_t = ip.tile([128, 1], mybir.dt.int32, tag="i0")
                i1_t = ip.tile([128, 1], mybir.dt.int32, tag="i1")
                nc.sync.dma_start(lg_t[:], lg[:, i, :])
                nc.sync.dma_start(m0_t[:], m0[:, i, :])
                nc.sync.dma_start(m1_t[:], m1[:, i, :])
                nc.sync.dma_start(i0_t[:], i0[:, i : i + 1])
                nc.sync.dma_start(i1_t[:], i1[:, i : i + 1])

                g0 = gp.tile([128, O], F32, tag="g0")
                g1 = gp.tile([128, O], F32, tag="g1")
                nc.gpsimd.indirect_dma_start(
                    out=g0[:],
                    out_offset=None,
                    in_=yall[:],
                    in_offset=bass.IndirectOffsetOnAxis(ap=i0_t[:, :1], axis=0),
                )
                nc.gpsimd.indirect_dma_start(
                    out=g1[:],
                    out_offset=None,
                    in_=yall[:],
                    in_offset=bass.IndirectOffsetOnAxis(ap=i1_t[:, :1], axis=0),
                )

                rmax = sp.tile([128, 1], F32, tag="rmax")
                nc.vector.reduce_max(out=rmax[:], in_=lg_t[:], axis=X)
                ex = sp.tile([128, E], F32, tag="ex")
                nc.vector.tensor_scalar_sub(out=ex[:], in0=lg_t[:], scalar1=rmax[:])
                nc.scalar.activation(
                    out=ex[:], in_=ex[:], func=mybir.ActivationFunctionType.Exp
                )
                s = sp.tile([128, 1], F32, tag="s")
                nc.vector.reduce_sum(out=s[:], in_=ex[:], axis=X)
                inv = sp.tile([128, 1], F32, tag="inv")
                nc.vector.reciprocal(out=inv[:], in_=s[:])

                t0 = sp.tile([128, E], F32, tag="t0")
                nc.vector.tensor_mul(out=t0[:], in0=ex[:], in1=m0_t[:])
                w0 = sp.tile([128, 1], F32, tag="w0")
                nc.vector.reduce_sum(out=w0[:], in_=t0[:], axis=X)
                nc.vector.tensor_mul(out=w0[:], in0=w0[:], in1=inv[:])

                t1 = sp.tile([128, E], F32, tag="t1")
                nc.vector.tensor_mul(out=t1[:], in0=ex[:], in1=m1_t[:])
                w1v = sp.tile([128, 1], F32, tag="w1")
                nc.vector.reduce_sum(out=w1v[:], in_=t1[:], axis=X)
                nc.vector.tensor_mul(out=w1v[:], in0=w1v[:], in1=inv[:])

                nc.vector.tensor_scalar_mul(out=g0[:], in0=g0[:], scalar1=w0[:])
                nc.vector.tensor_scalar_mul(out=g1[:], in0=g1[:], scalar1=w1v[:])
                nc.vector.tensor_add(out=g0[:], in0=g0[:], in1=g1[:])
                nc.sync.dma_start(out[:, i, :], g0[:])
    nc.compile()
    return nc


def kernel(x, W1, b1, W2, b2, W3, b3, Wg1, bg1, Wg2, bg2, top_k):
    x = np.asarray(x, np.float32)
    W1 = np.asarray(W1, np.float32)
    W2 = np.asarray(W2, np.float32)
    W3 = np.asarray(W3, np.float32)
    Wg1 = np.asarray(Wg1, np.float32)
    Wg2 = np.asarray(Wg2, np.float32)
    assert int(np.asarray(top_k)) == 2
    for b in (b1, b2, b3, bg1, bg2):
        assert not np.any(np.asarray(b)), "nonzero biases unsupported"

    core_ids = list(range(NCORES))

    # ---------------- L1: gating logits on device ----------------
    if "gate" not in _CACHE:
        _CACHE["gate"] = _build_gate_nc()
    nc1 = _CACHE["gate"]

    xT = np.ascontiguousarray(x.T)  # [D, N]
    wg1p = np.zeros((D, 128), np.float32)
    wg1p[:, :64] = Wg1
    wg2p = np.zeros((128, 128), np.float32)
    wg2p[:64, :E] = Wg2
    wg1_pmn = _to_pmn(wg1p)
    wg2_pmn = _to_pmn(wg2p)
    in1 = [
        {
            "xt": _to_pmn(xT[:, c * TPC : (c + 1) * TPC]),
            "wg1": wg1_pmn,
            "wg2": wg2_pmn,
        }
        for c in core_ids
    ]
    res1 = run_bass_kernel_spmd(nc1, in1, core_ids).results
    logits = np.concatenate(
        [res1[c]["logt"][:E, 0, :].T for c in core_ids], axis=0
    )  # [N, E]

    # ---------------- host routing (comparisons/indexing only) ----------------
    top2 = np.argsort(-logits, axis=1, kind="stable")[:, :2]  # [N, 2]
    e0s, e1s = top2[:, 0], top2[:, 1]
    expert_lists = [np.nonzero((top2 == e).any(axis=1))[0] for e in range(E)]
    counts = np.array([len(t) for t in expert_lists])

    # Balanced segmentation: chop each expert's tokens into <=S chunks, 16
    # uniform slots total, run the 8-core MLP module twice (8 slots/launch).
    NSEG = 2 * NCORES
    S = max(128, -(-int(counts.sum()) // (NSEG * 128)) * 128)
    while sum(-(-c // S) for c in counts if c) > NSEG:
        S += 128
    segs = []  # (expert, token_array)
    for e in range(E):
        tl = expert_lists[e]
        for j in range(0, max(len(tl), 1), S):
            segs.append((e, tl[j : j + S]))
    while len(segs) < NSEG:
        segs.append((0, np.zeros(0, np.int64)))

    # token -> global row in yall: segment k occupies rows [k*S, k*S+len_k)
    seg_of_expert = {}  # (expert, chunk_idx) -> seg index
    for k, (e, tl) in enumerate(segs):
        if len(tl):
            seg_of_expert.setdefault(e, []).append(k)
    pos = np.zeros((N, E), np.int64)
    for e in range(E):
        pos[expert_lists[e], e] = np.arange(len(expert_lists[e]))

    def glob_idx(es):
        p = pos[np.arange(N), es]
        segids = np.array(
            [seg_of_expert[e][pp // S] for e, pp in zip(es, p)], np.int64
        )
        return (segids * S + (p % S)).astype(np.int32)

    glob0, glob1 = glob_idx(e0s), glob_idx(e1s)
    eye = np.eye(E, dtype=np.float32)

    # ---------------- L2: per-expert MLP on device (2 launches) ----------------
    key2 = ("mlp", S)

    def seg_inputs(k):
        e, tl = segs[k]
        padded = np.zeros(S, np.int64)
        padded[: len(tl)] = tl
        Xe = x[padded]  # [S, D]
        return {
            "xt": _to_pmn(np.ascontiguousarray(Xe.T)),
            "w1": _to_pmn(W1[e]),
            "w2": _to_pmn(W2[e]),
            "w3": _to_pmn(W3[e]),
        }

    yall = np.empty((NSEG * S, O), np.float32)
    try:
        key2f = ("mlp2", S)
        if key2f not in _CACHE:
            _CACHE[key2f] = _build_mlp2_nc(S)
        in2 = []
        for c in core_ids:
            a, b = seg_inputs(c), seg_inputs(NCORES + c)
            in2.append(
                {**{k + "A": v for k, v in a.items()}, **{k + "B": v for k, v in b.items()}}
            )
        res2 = run_bass_kernel_spmd(_CACHE[key2f], in2, core_ids).results
        for c in core_ids:
            for half, s in enumerate(("A", "B")):
                k = half * NCORES + c
                yT = res2[c][f"yt{s}"].transpose(1, 0, 2).reshape(O, S)
                yall[k * S : (k + 1) * S] = yT.T
    except Exception:
        if key2 not in _CACHE:
            _CACHE[key2] = _build_mlp_nc(S)
        nc2 = _CACHE[key2]
        for half in range(2):
            in2 = [seg_inputs(half * NCORES + c) for c in core_ids]
            res2 = run_bass_kernel_spmd(nc2, in2, core_ids).results
            for c in core_ids:
                k = half * NCORES + c
                yT = res2[c]["yt"].transpose(1, 0, 2).reshape(O, S)
                yall[k * S : (k + 1) * S] = yT.T

    # ---------------- L3: gather + softmax combine on device ----------------
    key3 = ("comb", NSEG * S)
    if key3 not in _CACHE:
        _CACHE[key3] = _build_combine_nc(NSEG * S)
    nc3 = _CACHE[key3]

    ntiles = TPC // 128

    def _pt(a):  # [TPC, ...] -> [128, ntiles, ...] with token = i*128 + p
        return np.ascontiguousarray(
            a.reshape(ntiles, 128, *a.shape[1:]).transpose(1, 0, *range(2, a.ndim + 1))
        )

    in3 = []
    for c in core_ids:
        sl = slice(c * TPC, (c + 1) * TPC)
        in3.append(
            {
                "yall": yall,
                "lg": _pt(logits[sl]),
                "m0": _pt(eye[e0s[sl]]),
                "m1": _pt(eye[e1s[sl]]),
                "i0": _pt(glob0[sl].reshape(TPC, 1))[:, :, 0],
                "i1": _pt(glob1[sl].reshape(TPC, 1))[:, :, 0],
            }
        )
    res3 = run_bass_kernel_spmd(nc3, in3, core_ids).results
    out = np.concatenate(
        [res3[c]["out"].transpose(1, 0, 2).reshape(TPC, O) for c in core_ids], axis=0
    )
    return out



# revision 15
# speedup vs baseline: 1.3838x; 1.3838x over previous
"""MoE (8 experts, top-2) Trainium2 Bass kernel, expert-parallel over 8 cores.

Pipeline (all FLOPs on device):
  gate:  fp32 gating logits for all tokens (data-parallel over cores)
  host:  top-2 selection + balanced variable-size slot packing (indexing only)
  mlp:   fused 3-layer expert MLP per slot, bf16 matmuls, SBUF-resident
         intermediates, G variable-capacity slots per core
  comb:  per-token gather of its two expert rows + on-device softmax combine
"""

import itertools
import numpy as np
import ml_dtypes

import jax

jax.config.update("jax_compilation_cache_dir", "/tmp/jax_comp_cache")
jax.config.update("jax_persistent_cache_min_entry_size_bytes", -1)
jax.config.update("jax_persistent_cache_min_compile_time_secs", 0)

import concourse.bass as bass
import concourse.mybir as mybir
import concourse.tile as tile
from concourse import bacc
from concourse.bass_utils import run_bass_kernel_spmd

N, D, H, O, E = 8192, 1024, 2048, 1024, 8
NCORES = 8
TPC = N // NCORES
F32 = mybir.dt.float32
BF16 = mybir.dt.bfloat16
I32 = mybir.dt.int32
BF = ml_dtypes.bfloat16
CHUNK = 512

_CACHE = {}   # role -> compiled nc (exactly one module per role, one launch each)
_META = {}    # role -> build params


def _nc():
    return bacc.Bacc(None, target_bir_lowering=False, debug=True)


def _to_pmn(a, dt=np.float32):
    """[K, N] row-major -> [128, K/128, N] with row k = m*128 + p."""
    K, Nn = a.shape
    return np.ascontiguousarray(
        a.reshape(K // 128, 128, Nn).transpose(1, 0, 2).astype(dt)
    )


# ---------------------------------------------------------------- gate
def _build_gate_nc():
    """Slim fp32 gate: logits = relu(x @ Wg1) @ Wg2, feature-major."""
    nc = _nc()
    xt = nc.dram_tensor("xt", [128, D // 128, TPC], F32, kind="ExternalInput")
    wg1 = nc.dram_tensor("wg1", [128, D // 128, 64], F32, kind="ExternalInput")
    wg2 = nc.dram_tensor("wg2", [64, E], F32, kind="ExternalInput")
    logt = nc.dram_tensor("logt", [E, TPC], F32, kind="ExternalOutput")
    RELU = mybir.ActivationFunctionType.Relu
    COPY = mybir.ActivationFunctionType.Copy
    with tile.TileContext(nc) as tc:
        with (
            tc.tile_pool(name="xp", bufs=2) as xp,
            tc.tile_pool(name="wp", bufs=1) as wp,
            tc.tile_pool(name="hp", bufs=1) as hp,
            tc.tile_pool(name="pp", bufs=2, space="PSUM") as pp,
            tc.tile_pool(name="wup", bufs=1) as wup,
        ):
            wg1_t = wp.tile([128, D // 128, 64], F32, tag="wg1")
            wg2_t = wp.tile([64, E], F32, tag="wg2")
            nc.sync.dma_start(wg1_t[:], wg1[:])
            nc.sync.dma_start(wg2_t[:], wg2[:])
            # PE clock warmup while the first x chunk DMA lands (the sim's
            # p-state ramp needs ~3us of continuous PE busy to hit full clock)
            wu_w = wup.tile([128, 128], BF16, tag="wu_w")
            wu_a = wup.tile([128, 512], BF16, tag="wu_a")
            nc.any.memset(wu_w[:], 0.0)
            nc.any.memset(wu_a[:], 0.0)
            wu_ps = pp.tile([128, 512], F32, tag="wu_ps")
            for i in range(20):
                nc.tensor.matmul(wu_ps[:], wu_w[:], wu_a[:], start=(i == 0), stop=(i == 19))
            hid = hp.tile([64, TPC], F32, tag="hid")
            # hidden = relu(Wg1^T x); accumulate over 8 k-tiles, 512-wide psum
            for h in range(TPC // 512):
                ps = pp.tile([64, 512], F32, tag="ps")
                xc = xp.tile([128, D // 128, 512], F32, tag="xc")
                nc.sync.dma_start(xc[:], xt[:, :, h * 512 : (h + 1) * 512])
                for k in range(D // 128):
                    nc.tensor.matmul(
                        ps[:],
                        wg1_t[:, k, :],
                        xc[:, k, :],
                        start=(k == 0),
                        stop=(k == D // 128 - 1),
                    )
                nc.scalar.activation(hid[:, h * 512 : (h + 1) * 512], ps[:], RELU)
            # logits = Wg2^T hidden  (K=64, M=E)
            lg = hp.tile([E, TPC], F32, tag="lg")
            for h in range(TPC // 512):
                ps2 = pp.tile([E, 512], F32, tag="ps2")
                nc.tensor.matmul(
                    ps2[:],
                    wg2_t[:],
                    hid[:, h * 512 : (h + 1) * 512],
                    start=True,
                    stop=True,
                )
                nc.scalar.activation(lg[:, h * 512 : (h + 1) * 512], ps2[:], COPY)
            nc.sync.dma_start(logt[:], lg[:])
    nc.compile()
    return nc


# ---------------------------------------------------------------- fused MLP
def _build_mlp_nc(caps):
    """Fused 3-layer MLP over G slots with capacities `caps` (tokens each,
    multiples of 128). Per slot: h1=relu(W1^T x), h2=relu(W2^T h1),
    y=W3^T h2, all bf16 matmuls w/ fp32 PSUM, h1/h2 SBUF-resident per
    512-token chunk. Feature-major layout: partition=feature, free=token."""
    nc = _nc()
    G = len(caps)
    KD, KH = D // 128, H // 128  # 8, 16
    MH, MO = H // 128, O // 128  # 16, 8
    RELU = mybir.ActivationFunctionType.Relu
    t = {}
    for g, S in enumerate(caps):
        t[f"x{g}"] = nc.dram_tensor(f"x{g}", [128, KD, S], BF16, kind="ExternalInput")
        t[f"w1{g}"] = nc.dram_tensor(f"w1{g}", [128, KD, H], BF16, kind="ExternalInput")
        t[f"w2{g}"] = nc.dram_tensor(f"w2{g}", [128, KH, H], BF16, kind="ExternalInput")
        t[f"w3{g}"] = nc.dram_tensor(f"w3{g}", [128, KH, O], BF16, kind="ExternalInput")
        t[f"y{g}"] = nc.dram_tensor(f"y{g}", [128, MO, S], BF16, kind="ExternalOutput")

    with tile.TileContext(nc) as tc:
        with (
            tc.tile_pool(name="wu", bufs=1) as wup,
            tc.tile_pool(name="w1r", bufs=1) as w1p,
            tc.tile_pool(name="w2r", bufs=1) as w2p,
            tc.tile_pool(name="w3r", bufs=1) as w3p,
            tc.tile_pool(name="xcp", bufs=2) as xp,
            tc.tile_pool(name="h1", bufs=1) as h1p,
            tc.tile_pool(name="h2", bufs=1) as h2p,
            tc.tile_pool(name="yc", bufs=2) as yp,
            tc.tile_pool(name="ps", bufs=8, space="PSUM") as pp,
        ):
            # PE clock warmup: junk bf16 matmuls, no input deps — keeps the
            # tensor engine continuously busy while the first DMAs land so the
            # sim's p-state ramp reaches full clock before real work starts.
            wu_w = wup.tile([128, 128], BF16, tag="wu_w")
            wu_a = wup.tile([128, 512], BF16, tag="wu_a")
            nc.any.memset(wu_w[:], 0.0)
            nc.any.memset(wu_a[:], 0.0)
            wu_ps = pp.tile([128, 512], F32, tag="ps")
            for i in range(24):
                nc.tensor.matmul(wu_ps[:], wu_w[:], wu_a[:], start=(i == 0), stop=(i == 23))
            for g, S in enumerate(caps):
                nchunks = (S + CHUNK - 1) // CHUNK
                # Slot-resident weights. All input DMAs go on the SP queue in
                # careful order (x0, w1, x1, w2, w3, x2, ...): outputs go on
                # DVE so they never head-of-line-block the next inputs.
                w1_t = w1p.tile([128, KD, H], BF16, tag="w1")
                w2_t = w2p.tile([128, KH, H], BF16, tag="w2")
                w3_t = w3p.tile([128, KH, O], BF16, tag="w3")
                xcs = []
                for c in range(nchunks):
                    o = c * CHUNK
                    cw = min(CHUNK, S - o)
                    xc = xp.tile([128, KD, CHUNK], BF16, tag="xc", name=f"xc{c}")
                    xcs.append(xc)
                    if c == 0:
                        nc.sync.dma_start(xc[:, :, :cw], t[f"x{g}"][:, :, o : o + cw])
                        for q in range(4):  # w1 in H-quarters: unlock L1 m-tiles early
                            nc.sync.dma_start(
                                w1_t[:, :, q * 512 : (q + 1) * 512],
                                t[f"w1{g}"][:, :, q * 512 : (q + 1) * 512],
                            )
                    elif c == 1:
                        nc.sync.dma_start(xc[:, :, :cw], t[f"x{g}"][:, :, o : o + cw])
                        for q in range(4):
                            nc.sync.dma_start(
                                w2_t[:, :, q * 512 : (q + 1) * 512],
                                t[f"w2{g}"][:, :, q * 512 : (q + 1) * 512],
                            )
                        for q in range(4):
                            nc.sync.dma_start(
                                w3_t[:, :, q * 256 : (q + 1) * 256],
                                t[f"w3{g}"][:, :, q * 256 : (q + 1) * 256],
                            )
                if nchunks == 1:
                    for q in range(4):
                        nc.sync.dma_start(
                            w2_t[:, :, q * 512 : (q + 1) * 512],
                            t[f"w2{g}"][:, :, q * 512 : (q + 1) * 512],
                        )
                    for q in range(4):
                        nc.sync.dma_start(
                            w3_t[:, :, q * 256 : (q + 1) * 256],
                            t[f"w3{g}"][:, :, q * 256 : (q + 1) * 256],
                        )
                for c in range(nchunks):
                    o = c * CHUNK
                    cw = min(CHUNK, S - o)
                    xc = xcs[c]
                    if c >= 2:
                        nc.sync.dma_start(xc[:, :, :cw], t[f"x{g}"][:, :, o : o + cw])
                    # ---- L1: h1 = relu(W1^T x) ----
                    h1ts = [h1p.tile([128, CHUNK], BF16, tag=f"h1_{m}", name=f"h1_{m}") for m in range(MH)]
                    for m in range(MH):
                        ps = pp.tile([128, CHUNK], F32, tag="ps")
                        for k in range(KD):
                            nc.tensor.matmul(
                                ps[:, :cw],
                                w1_t[:, k, m * 128 : (m + 1) * 128],
                                xc[:, k, :cw],
                                start=(k == 0),
                                stop=(k == KD - 1),
                            )
                        nc.scalar.activation(h1ts[m][:, :cw], ps[:, :cw], RELU)
                    # ---- L2: h2 = relu(W2^T h1) ----
                    h2ts = [h2p.tile([128, CHUNK], BF16, tag=f"h2_{m}", name=f"h2_{m}") for m in range(MH)]
                    for m in range(MH):
                        ps = pp.tile([128, CHUNK], F32, tag="ps")
                        for k in range(KH):
                            nc.tensor.matmul(
                                ps[:, :cw],
                                w2_t[:, k, m * 128 : (m + 1) * 128],
                                h1ts[k][:, :cw],
                                start=(k == 0),
                                stop=(k == KH - 1),
                            )
                        nc.scalar.activation(h2ts[m][:, :cw], ps[:, :cw], RELU)
                    # ---- L3: y = W3^T h2 (drain + store on DVE) ----
                    for m in range(MO):
                        ps = pp.tile([128, CHUNK], F32, tag="ps")
                        for k in range(KH):
                            nc.tensor.matmul(
                                ps[:, :cw],
                                w3_t[:, k, m * 128 : (m + 1) * 128],
                                h2ts[k][:, :cw],
                                start=(k == 0),
                                stop=(k == KH - 1),
                            )
                        yt = yp.tile([128, CHUNK], BF16, tag=f"y_{m % 2}", name=f"y_{m % 2}")
                        nc.vector.tensor_copy(out=yt[:, :cw], in_=ps[:, :cw])
                        nc.gpsimd.dma_start(t[f"y{g}"][:, m, o : o + cw], yt[:, :cw])
    nc.compile()
    return nc


# ---------------------------------------------------------------- combine
def _build_comb_nc(yall_rows):
    """Gather each token's two expert rows (bf16) + softmax combine -> fp32."""
    nc = _nc()
    ntiles = TPC // 128
    yall = nc.dram_tensor("yall", [yall_rows, O], BF16, kind="ExternalInput")
    lg = nc.dram_tensor("lg", [128, ntiles, E], F32, kind="ExternalInput")
    m0 = nc.dram_tensor("m0", [128, ntiles, E], F32, kind="ExternalInput")
    m1 = nc.dram_tensor("m1", [128, ntiles, E], F32, kind="ExternalInput")
    i0 = nc.dram_tensor("i0", [128, ntiles], I32, kind="ExternalInput")
    i1 = nc.dram_tensor("i1", [128, ntiles], I32, kind="ExternalInput")
    out = nc.dram_tensor("out", [128, ntiles, O], F32, kind="ExternalOutput")
    X = mybir.AxisListType.X
    EXP = mybir.ActivationFunctionType.Exp
    COPY = mybir.ActivationFunctionType.Copy
    with tile.TileContext(nc) as tc:
        with (
            tc.tile_pool(name="big", bufs=4) as gp,
            tc.tile_pool(name="acc", bufs=4) as ap,
            tc.tile_pool(name="small", bufs=2) as sp,
            tc.tile_pool(name="idx", bufs=2) as ip,
        ):
            lg_t = sp.tile([128, ntiles, E], F32, tag="lg")
            m0_t = sp.tile([128, ntiles, E], F32, tag="m0")
            m1_t = sp.tile([128, ntiles, E], F32, tag="m1")
            i0_t = ip.tile([128, ntiles], I32, tag="i0")
            i1_t = ip.tile([128, ntiles], I32, tag="i1")
            nc.sync.dma_start(lg_t[:], lg[:])
            nc.sync.dma_start(m0_t[:], m0[:])
            nc.sync.dma_start(m1_t[:], m1[:])
            nc.sync.dma_start(i0_t[:], i0[:])
            nc.sync.dma_start(i1_t[:], i1[:])
            for i in range(ntiles):
                g0 = gp.tile([128, O], BF16, tag="g0")
                g1 = gp.tile([128, O], BF16, tag="g1")
                nc.gpsimd.indirect_dma_start(
                    out=g0[:],
                    out_offset=None,
                    in_=yall[:],
                    in_offset=bass.IndirectOffsetOnAxis(ap=i0_t[:, i : i + 1], axis=0),
                )
                nc.gpsimd.indirect_dma_start(
                    out=g1[:],
                    out_offset=None,
                    in_=yall[:],
                    in_offset=bass.IndirectOffsetOnAxis(ap=i1_t[:, i : i + 1], axis=0),
                )
                # softmax weights from logits (original gate values of top-2)
                rmax = sp.tile([128, 1], F32, tag="rmax")
                nc.vector.reduce_max(out=rmax[:], in_=lg_t[:, i, :], axis=X)
                ex = sp.tile([128, E], F32, tag="ex")
                nc.vector.tensor_scalar_sub(out=ex[:], in0=lg_t[:, i, :], scalar1=rmax[:])
                nc.scalar.activation(out=ex[:], in_=ex[:], func=EXP)
                s = sp.tile([128, 1], F32, tag="s")
                nc.vector.reduce_sum(out=s[:], in_=ex[:], axis=X)
                inv = sp.tile([128, 1], F32, tag="inv")
                nc.vector.reciprocal(out=inv[:], in_=s[:])
                t0 = sp.tile([128, E], F32, tag="t0")
                nc.vector.tensor_mul(out=t0[:], in0=ex[:], in1=m0_t[:, i, :])
                w0 = sp.tile([128, 1], F32, tag="w0")
                nc.vector.reduce_sum(out=w0[:], in_=t0[:], axis=X)
                nc.vector.tensor_mul(out=w0[:], in0=w0[:], in1=inv[:])
                t1 = sp.tile([128, E], F32, tag="t1")
                nc.vector.tensor_mul(out=t1[:], in0=ex[:], in1=m1_t[:, i, :])
                w1v = sp.tile([128, 1], F32, tag="w1")
                nc.vector.reduce_sum(out=w1v[:], in_=t1[:], axis=X)
                nc.vector.tensor_mul(out=w1v[:], in0=w1v[:], in1=inv[:])
                # out = g0*w0 + g1*w1   (ACT for the scaled copies, DVE adds)
                a0 = ap.tile([128, O], F32, tag="a0")
                a1 = ap.tile([128, O], F32, tag="a1")
                nc.scalar.activation(out=a0[:], in_=g0[:], func=COPY, scale=w0[:])
                nc.scalar.activation(out=a1[:], in_=g1[:], func=COPY, scale=w1v[:])
                nc.vector.tensor_add(out=a0[:], in0=a0[:], in1=a1[:])
                nc.sync.dma_start(out[:, i, :], a0[:])
    nc.compile()
    return nc


# ---------------------------------------------------------------- packing
def _cover_options(sizes, avail, need, max_waste=512, cap=24):
    """Multisets over `sizes` (resp. avail) with need <= sum <= need+max_waste,
    smallest waste first; each option is a dict size->n."""
    opts = []
    hi = need + max_waste

    def rec(i, rem, tot, cur):
        if len(opts) > 500:
            return
        if rem <= 0:
            opts.append((tot - need, dict(cur)))
            return
        if i == len(sizes):
            return
        s = sizes[i]
        maxn = min(avail[s], -(-rem // s), (hi - tot + rem - 1) // s if s else 0)
        maxn = min(maxn, (hi - (tot - 0)) // s + 1)
        for n in range(maxn, -1, -1):
            if tot + s * n > hi:
                continue
            if n:
                cur[s] = n
            rec(i + 1, rem - s * n, tot + s * n, cur)
            cur.pop(s, None)

    rec(0, need, 0, {})
    opts.sort(key=lambda o: (o[0], sum(o[1].values())))
    return opts[:cap]


def _feasible(patt, counts, ncores, max_waste):
    """Backtracking cover: assign slot multisets to experts, col sums bounded."""
    avail = {}
    for s in patt:
        avail[s] = avail.get(s, 0) + ncores
    sizes = sorted(avail, reverse=True)
    order = sorted(range(len(counts)), key=lambda e: -counts[e])
    assign = {e: [] for e in range(len(counts))}
    calls = [0]

    def bt(i, waste_left):
        calls[0] += 1
        if calls[0] > 20000:
            return False
        if i == len(order):
            return True
        e = order[i]
        if counts[e] == 0:
            return bt(i + 1, waste_left)
        for w, opt in _cover_options(sizes, avail, counts[e], waste_left):
            for s, n in opt.items():
                avail[s] -= n
            assign[e] = [s for s, n in opt.items() for _ in range(n)]
            if bt(i + 1, waste_left - w):
                return True
            for s, n in opt.items():
                avail[s] += n
        assign[e] = []
        return False

    return assign if bt(0, max_waste) else None


def _solve_pattern(counts, ncores=8, budget_s=60.0):
    """Pick per-core slot capacities (same pattern on every core, SPMD)."""
    import time as _time

    t0 = _time.time()
    sizes = [c * 128 for c in range(1, 13)]  # 128 .. 1536
    lo = -(-(sum(counts)) // (ncores * 128)) * 128
    cands = []
    for G in (3, 4, 5):
        for patt in itertools.combinations_with_replacement(sorted(sizes, reverse=True), G):
            tot = sum(patt)
            if lo <= tot <= lo + 512:
                cands.append((tot, G, patt))
    cands.sort()
    for tot, G, patt in cands:
        if _time.time() - t0 > budget_s:
            break
        max_waste = ncores * tot - sum(counts)
        asg = _feasible(patt, counts, ncores, max_waste)
        if asg is not None:
            return patt, asg
    # fallback: generous uniform pattern
    S = -(-sum(counts) // (ncores * 2 * 128)) * 128
    while True:
        patt = (S, S)
        asg = _feasible(patt, counts, ncores, 10 * 128 * ncores)
        if asg is not None:
            return patt, asg
        S += 128


# ---------------------------------------------------------------- driver
def kernel(x, W1, b1, W2, b2, W3, b3, Wg1, bg1, Wg2, bg2, top_k):
    x = np.asarray(x, np.float32)
    W1 = np.asarray(W1, np.float32)
    W2 = np.asarray(W2, np.float32)
    W3 = np.asarray(W3, np.float32)
    Wg1 = np.asarray(Wg1, np.float32)
    Wg2 = np.asarray(Wg2, np.float32)
    assert int(np.asarray(top_k)) == 2
    for b in (b1, b2, b3, bg1, bg2):
        assert not np.any(np.asarray(b)), "nonzero biases unsupported"

    core_ids = list(range(NCORES))

    # ---------------- gate: fp32 logits on device ----------------
    if "gate" not in _CACHE:
        _CACHE["gate"] = _build_gate_nc()
    nc1 = _CACHE["gate"]

    xT = np.ascontiguousarray(x.T)  # [D, N]
    wg1_pmn = _to_pmn(Wg1)  # [128, 8, 64]
    wg2_in = np.ascontiguousarray(Wg2, np.float32)  # [64, E]
    in1 = [
        {
            "xt": _to_pmn(xT[:, c * TPC : (c + 1) * TPC]),
            "wg1": wg1_pmn,
            "wg2": wg2_in,
        }
        for c in core_ids
    ]
    res1 = run_bass_kernel_spmd(nc1, in1, core_ids).results
    logits = np.concatenate(
        [res1[c]["logt"].T for c in core_ids], axis=0
    )  # [N, E] fp32

    # ---------------- host routing (comparisons/indexing only) ----------------
    top2 = np.argsort(-logits, axis=1, kind="stable")[:, :2]
    e0s, e1s = top2[:, 0], top2[:, 1]
    expert_lists = [np.nonzero((top2 == e).any(axis=1))[0] for e in range(E)]
    counts = [len(t) for t in expert_lists]

    patt, assign = _solve_pattern(counts, NCORES)
    G = len(patt)

    # place parts onto (core, slot_pos): group slot positions by size
    pos_by_size = {}
    for j, s in enumerate(patt):
        pos_by_size.setdefault(s, []).append(j)
    free_by_size = {s: [(c, j) for j in js for c in range(NCORES)]
                    for s, js in pos_by_size.items()}
    # slot table: (core, pos) -> (expert, token_array)
    slot_tok = {}
    tok_row = np.zeros(N, np.int64)  # token -> row within its expert stream
    part_of = {e: [] for e in range(E)}
    for e in range(E):
        off = 0
        for s in assign[e]:
            c, j = free_by_size[s].pop()
            toks = expert_lists[e][off : off + s]
            slot_tok[(c, j)] = (e, toks)
            part_of[e].append((off, s, c, j))
            off += s

    # global row base per slot: order = core-major, slot-pos minor
    caps_prefix = np.concatenate([[0], np.cumsum(patt)])
    per_core = caps_prefix[-1]
    R = NCORES * per_core

    def slot_base(c, j):
        return c * per_core + caps_prefix[j]

    # token -> global row for each of its two experts
    glob = np.zeros((N, 2), np.int64)
    pos_in_expert = np.zeros((N, E), np.int64)
    for e in range(E):
        pos_in_expert[expert_lists[e], e] = np.arange(counts[e])
    for which, es in enumerate((e0s, e1s)):
        p = pos_in_expert[np.arange(N), es]
        base = np.zeros(N, np.int64)
        loc = np.zeros(N, np.int64)
        for e in range(E):
            sel = es == e
            pe = p[sel]
            b = np.zeros(len(pe), np.int64)
            l = np.zeros(len(pe), np.int64)
            for off, s, c, j in part_of[e]:
                m = (pe >= off) & (pe < off + s)
                b[m] = slot_base(c, j)
                l[m] = pe[m] - off
            base[sel] = b
            loc[sel] = l
        glob[:, which] = base + loc

    # ---------------- mlp launch ----------------
    if _META.get("mlp") != patt:
        _CACHE["mlp"] = _build_mlp_nc(patt)
        _META["mlp"] = patt
    nc2 = _CACHE["mlp"]

    xbf = x.astype(BF)
    w_pmn = {}  # (layer, e) -> pmn bf16

    def wp(layer, W, e):
        key = (layer, e)
        if key not in w_pmn:
            w_pmn[key] = _to_pmn(W[e], BF)
        return w_pmn[key]

    zero_w = {
        1: np.zeros((128, D // 128, H), BF),
        2: np.zeros((128, H // 128, H), BF),
        3: np.zeros((128, H // 128, O), BF),
    }
    in2 = []
    for c in core_ids:
        m = {}
        for j, s in enumerate(patt):
            e, toks = slot_tok.get((c, j), (None, None))
            xg = np.zeros((128, D // 128, s), BF)
            if e is not None and len(toks):
                xe = np.ascontiguousarray(xbf[toks].T)  # [D, n]
                xg[:, :, : len(toks)] = xe.reshape(D // 128, 128, len(toks)).transpose(1, 0, 2)
            m[f"x{j}"] = xg
            if e is None:
                m[f"w1{j}"], m[f"w2{j}"], m[f"w3{j}"] = zero_w[1], zero_w[2], zero_w[3]
            else:
                m[f"w1{j}"] = wp(1, W1, e)
                m[f"w2{j}"] = wp(2, W2, e)
                m[f"w3{j}"] = wp(3, W3, e)
        in2.append(m)
    res2 = run_bass_kernel_spmd(nc2, in2, core_ids).results

    yall = np.zeros((R, O), BF)
    for c in core_ids:
        for j, s in enumerate(patt):
            yt = res2[c][f"y{j}"]  # [128, O//128, s] bf16
            b = slot_base(c, j)
            yall[b : b + s] = yt.transpose(1, 0, 2).reshape(O, s).T

    # ---------------- combine launch ----------------
    if _META.get("comb") != R:
        _CACHE["comb"] = _build_comb_nc(R)
        _META["comb"] = R
    nc3 = _CACHE["comb"]

    ntiles = TPC // 128
    eye = np.eye(E, dtype=np.float32)

    def _pt(a):  # [TPC, ...] -> [128, ntiles, ...] with token = i*128 + p
        return np.ascontiguousarray(
            a.reshape(ntiles, 128, *a.shape[1:]).transpose(1, 0, *range(2, a.ndim + 1))
        )

    g0 = glob[:, 0].astype(np.int32)
    g1 = glob[:, 1].astype(np.int32)
    in3 = []
    for c in core_ids:
        sl = slice(c * TPC, (c + 1) * TPC)
        in3.append(
            {
                "yall": yall,
                "lg": _pt(logits[sl]),
                "m0": _pt(eye[e0s[sl]]),
                "m1": _pt(eye[e1s[sl]]),
                "i0": _pt(g0[sl].reshape(TPC, 1))[:, :, 0],
                "i1": _pt(g1[sl].reshape(TPC, 1))[:, :, 0],
            }
        )
    res3 = run_bass_kernel_spmd(nc3, in3, core_ids).results
    out = np.concatenate(
        [res3[c]["out"].transpose(1, 0, 2).reshape(TPC, O) for c in core_ids], axis=0
    )
    return out
